# revision 25
# baseline (speedup 1.0000x reference)
"""Trainium2 Bass kernel for the Ergodicity loss.

loss = sum_b sum_pq ((S[b,p,q]/(nf*N*T) - cd[p,q])^2 * nw[p,q])
       + 1e-3 * sum(u^2) / (2*N*T*B)
where S[b,p,q] = sum_{t,n} cos(p*pi*x0) * cos(q*pi*x1)     (L == 1)

Strategy (8 cores, data-parallel over batch B=32 -> 4 per core):
  * ACT computes s1 = sin(pi x), c1 = cos(pi x) (inside Sin's valid
    range); DVE runs the Chebyshev recurrence s_k = 2 c1 s_{k-1} -
    s_{k-2} in fp16 (2x perf-mode tensor_tensor only).
  * cos identities: cos(2m t) = 1 - 2 s_m^2, cos((2i+1) t) = c1 -
    2 s_{i+1} s_i.  The Gram matmul therefore runs over RAW feature
    columns (bf16): one shared ones-column, and per batch element
    {c1, s_1^2..s_15^2, s_2 s_1, ..., s_16 s_15} (125 used + 3 zero
    pads).  Squares come from the otherwise-idle ScalarE (Square
    activation, stride-insensitive); odd products are single fp16
    tensor_tensor ops on DVE writing d-interleaved pairs (2x mode).
  * C layout col = (s*128 + c)*2 + d (s = sample column (jl n), c =
    function, d = dim) makes every matmul operand a 4-byte-stride AP
    (measured as fast as contiguous).  256 matmuls [128,128]x[128,128]
    bf16 accumulate into 2 alternating PSUM banks.
  * true S = A G A^T with sparse A (host, microseconds) + final loss.
  * u^2 on ScalarE (Square with accum_out); host sums the column.
"""

import math
from contextlib import ExitStack

import numpy as np

import concourse.bass as bass
import concourse.bacc as bacc
import concourse.mybir as mybir
import concourse.tile as tile
from concourse.bass_utils import run_bass_kernel_spmd

T, B, N, D, K = 512, 32, 64, 2, 32
NCORES = 8
BL = B // NCORES            # 4 batch elements per core
NT = N * T                  # 32768 samples per batch element
JJ = T // 128               # 4 t-chunks of 128 partitions
SCOL = 2 * N                # 128 sample columns (jl, n) per j-half
HCOLS = BL * SCOL * D       # 1024 x-columns per j-half (b, jl n, d)
NC = 128                    # function columns in the Gram
CTRL_SCALE = 1e-3 / (2.0 * N * T * B)
SAFETY = 1.0 - 1e-6         # keeps Sin's argument strictly inside [-pi, pi]

f32 = mybir.dt.float32
fp16 = mybir.dt.float16
bf16 = mybir.dt.bfloat16
ALU = mybir.AluOpType
ACTF = mybir.ActivationFunctionType

LAST_RESULTS = None         # stashed BassKernelResults for test harnesses


def colid(p, b):
    """Gram column index of cos-mode p for batch-slot b (device + host)."""
    if p == 0:
        return 0                      # shared ones column
    i = 1 + 31 * b
    if p == 1:
        return i                      # c1
    if p % 2 == 0:
        return i + p // 2             # s_m^2, m = p/2 in 1..15
    return i + 15 + (p - 1) // 2      # s_{i+1} s_i, i = (p-1)/2 in 1..15


def _build_body(ctx, tc, x_h, u_h, ga_h, gb_h, uc_h):
    nc = tc.nc

    xpool = ctx.enter_context(tc.tile_pool(name="xp", bufs=1))
    cpool = ctx.enter_context(tc.tile_pool(name="cp", bufs=1))
    spool = ctx.enter_context(tc.tile_pool(name="sp", bufs=6))
    qpool = ctx.enter_context(tc.tile_pool(name="qp", bufs=3))
    mpool = ctx.enter_context(tc.tile_pool(name="mp", bufs=1))
    ppool = ctx.enter_context(tc.tile_pool(name="pp", bufs=1, space="PSUM"))

    # ---- inputs to SBUF ----
    # x[t, b, n, d] -> X_h[p = t%128, (b (jl n) d)] for the two j-halves
    xv = x_h[:].rearrange("(j p) b n d -> p b j (n d)", j=JJ, p=128)
    Xh = []
    for h in range(2):
        X = xpool.tile([128, HCOLS], f32, tag=f"x{h}")
        nc.sync.dma_start(
            X[:].rearrange("p (b jl nd) -> p b jl nd", b=BL, jl=2, nd=N * D),
            xv[:, :, 2 * h : 2 * h + 2, :],
        )
        Xh.append(X)

    U = xpool.tile([128, 2048], f32, tag="u")
    nc.sync.dma_start(U[:], u_h[:].rearrange("(p a) b n d -> p (a b n d)", p=128))

    sc = mpool.tile([128, 8], f32, tag="scratch")
    bias_c1 = sc[:, 0:1]
    nc.vector.memset(bias_c1, float(np.float32(math.pi / 2 * SAFETY)))

    # u^2 summed per partition on DVE (early: fills the DMA-wait window)
    udum = mpool.tile([128, 2048], f32, tag="udum")
    ucol = sc[:, 1:2]
    nc.vector.tensor_mul(udum[:], U[:], U[:])
    nc.vector.tensor_reduce(ucol, udum[:], mybir.AxisListType.X, ALU.add)
    nc.sync.dma_start(uc_h[:], ucol)

    # ---- feature-column tensors: C_h[p, (s c d)], bf16 ----
    Ch = []
    for h in range(2):
        C = cpool.tile([128, NC * SCOL * D], bf16, tag=f"c{h}")
        CW = C[:].rearrange("p (s c d) -> p c s d", s=SCOL, c=NC, d=D)
        nc.vector.memset(CW[:, 0], 1.0)               # shared ones column
        nc.vector.memset(CW[:, 125:128], 0.0)         # zero pads
        Ch.append(C)

    Ga = ppool.tile([128, 128], f32, tag="ga")
    Gb = ppool.tile([128, 128], f32, tag="gb")

    mma = 0
    mmb = 0
    for h in range(2):
        X, C = Xh[h], Ch[h]

        # per-b column-family view: [p, i(31), b, s, d] for c = 1 + 31 b + i
        CF = C[:].rearrange("p (s c d) -> p s c d", s=SCOL, c=NC, d=D)
        CF = CF[:, :, 1:125, :].rearrange("p s (b i) d -> p i b s d", b=BL, i=31)

        def fcol(i):
            return CF[:, i]           # [128, b, s, d]

        Xin = X[:].rearrange("p (b s d) -> p b s d", b=BL, s=SCOL, d=D)

        def s_in(t):
            return t[:].rearrange("p (b s d) -> p b s d", b=BL, s=SCOL, d=D)

        # c1: fp16 tile for the chain + bf16 columns (both on ACT)
        c1 = qpool.tile([128, HCOLS], fp16, tag="c1")
        nc.scalar.activation(c1[:], X[:], ACTF.Sin,
                             bias=bias_c1, scale=float(np.float32(-math.pi * SAFETY)))
        nc.vector.tensor_copy(fcol(0), c1[:].rearrange(
            "p (b s d) -> p b s d", b=BL, s=SCOL, d=D))

        s_prev = spool.tile([128, HCOLS], fp16, tag="s")   # s_1
        nc.scalar.activation(s_prev[:], X[:], ACTF.Sin,
                             bias=0.0, scale=float(np.float32(math.pi * SAFETY)))

        c1d = qpool.tile([128, HCOLS], fp16, tag="c1d")    # 2*c1
        nc.vector.tensor_scalar_mul(c1d[:], c1[:], 2.0)

        # s_2 = 2 s_1 c_1 ; then per mode: squares on ACT, products on DVE
        s_cur = spool.tile([128, HCOLS], fp16, tag="s")
        nc.vector.tensor_mul(s_cur[:], s_prev[:], c1d[:])
        nc.vector.tensor_mul(fcol(1), s_in(s_prev), s_in(s_prev))    # s_1^2
        nc.vector.tensor_mul(fcol(16), s_in(s_cur), s_in(s_prev))    # s_2 s_1
        s_prev2, s_prev = s_prev, s_cur

        for m in range(3, 17):
            # s_m = 2 c1 s_{m-1} - s_{m-2}
            q = qpool.tile([128, HCOLS], fp16, tag="q")
            nc.vector.tensor_mul(q[:], s_prev[:], c1d[:])
            s_cur = spool.tile([128, HCOLS], fp16, tag="s")
            nc.vector.tensor_sub(s_cur[:], q[:], s_prev2[:])
            if m - 1 <= 15:
                nc.vector.tensor_mul(fcol(m - 1), s_in(s_prev), s_in(s_prev))
            nc.vector.tensor_mul(fcol(15 + m - 1), s_in(s_cur), s_in(s_prev))
            s_prev2, s_prev = s_prev, s_cur

        # Gram matmuls: one per sample column, alternating PSUM banks
        CM = C[:].rearrange("p (s c d) -> p s d c", s=SCOL, c=NC, d=D)
        for s_i in range(SCOL):
            if (s_i % 2) == 0:
                nc.tensor.matmul(Ga[:, :], CM[:, s_i, 0], CM[:, s_i, 1],
                                 start=(mma == 0), stop=(mma == JJ * N // 2 - 1))
                mma += 1
            else:
                nc.tensor.matmul(Gb[:, :], CM[:, s_i, 0], CM[:, s_i, 1],
                                 start=(mmb == 0), stop=(mmb == JJ * N // 2 - 1))
                mmb += 1

    # ---- outputs ----
    ga_sb = mpool.tile([128, 128], f32, tag="gasb")
    gb_sb = mpool.tile([128, 128], f32, tag="gbsb")
    nc.vector.tensor_copy(ga_sb[:], Ga[:, :])
    nc.vector.tensor_copy(gb_sb[:], Gb[:, :])
    nc.sync.dma_start(ga_h[:], ga_sb[:])
    nc.sync.dma_start(gb_h[:], gb_sb[:])


def _build_nc():
    nc = bacc.Bacc()
    x_h = nc.declare_dram_parameter("x", [T, BL, N, D], f32, isOutput=False)
    u_h = nc.declare_dram_parameter("u", [T, BL, N, D], f32, isOutput=False)
    ga_h = nc.declare_dram_parameter("ga", [128, 128], f32, isOutput=True)
    gb_h = nc.declare_dram_parameter("gb", [128, 128], f32, isOutput=True)
    uc_h = nc.declare_dram_parameter("uc", [128, 1], f32, isOutput=True)
    with tile.TileContext(nc) as tc:
        with ExitStack() as ctx:
            _build_body(ctx, tc, x_h, u_h, ga_h, gb_h, uc_h)
    nc.finalize()
    return nc


_NC_CACHE = None


def _get_nc():
    global _NC_CACHE
    if _NC_CACHE is None:
        _NC_CACHE = _build_nc()
    return _NC_CACHE


def _amat(b):
    """A[p, col]: cos-mode p as a linear combo of raw Gram columns."""
    A = np.zeros((K, NC), np.float32)
    for p in range(K):
        if p == 0:
            A[p, 0] = 1.0
        elif p == 1:
            A[p, colid(1, b)] = 1.0
        elif p % 2 == 0:
            A[p, colid(p, b)] = -2.0
            A[p, 0] += 1.0                     # + ones
        else:
            A[p, colid(p, b)] = -2.0
            A[p, colid(1, b)] += 1.0           # + c1
    return A


_AMATS = [_amat(b) for b in range(BL)]


def host_loss(gs, ucols, coeffs_density, norm_factors, norm_weights):
    nf = np.asarray(norm_factors, np.float32)
    cd = np.asarray(coeffs_density, np.float32)
    nw = np.asarray(norm_weights, np.float32)
    total = np.float32(0.0)
    for G, ucol in zip(gs, ucols):
        for b in range(BL):
            A = _AMATS[b]
            Sp = (A @ G @ A.T).astype(np.float32)
            coeffs = Sp / (nf * np.float32(NT))
            total = np.float32(
                total + (((coeffs - cd) ** 2) * nw).sum(dtype=np.float32))
        total = np.float32(
            total + np.float32(CTRL_SCALE) * ucol.sum(dtype=np.float32))
    return np.float32(total)


def make_in_maps(x, u):
    x = np.ascontiguousarray(np.asarray(x, dtype=np.float32))
    u = np.ascontiguousarray(np.asarray(u, dtype=np.float32))
    in_maps = []
    for c in range(NCORES):
        in_maps.append({
            "x": np.ascontiguousarray(x[:, BL * c : BL * (c + 1)]),
            "u": np.ascontiguousarray(u[:, BL * c : BL * (c + 1)]),
        })
    return in_maps


def kernel(x, u, L, coeffs_density, norm_factors, norm_weights):
    global LAST_RESULTS
    nc = _get_nc()
    in_maps = make_in_maps(x, u)
    res = run_bass_kernel_spmd(nc, in_maps, list(range(NCORES)))
    LAST_RESULTS = res
    gs = [np.asarray(r["ga"], np.float32) + np.asarray(r["gb"], np.float32)
          for r in res.results]
    ucols = [np.asarray(r["uc"], np.float32) for r in res.results]
    return host_loss(gs, ucols, coeffs_density, norm_factors, norm_weights)


# revision 28
# speedup vs baseline: 1.0106x; 1.0106x over previous
"""Trainium2 Bass kernel for the Ergodicity loss.

loss = sum_b sum_pq ((S[b,p,q]/(nf*N*T) - cd[p,q])^2 * nw[p,q])
       + 1e-3 * sum(u^2) / (2*N*T*B)
where S[b,p,q] = sum_{t,n} cos(p*pi*x0) * cos(q*pi*x1)     (L == 1)

Strategy (8 cores, data-parallel over batch B=32 -> 4 per core):
  * ACT computes s1 = sin(pi x), c1 = cos(pi x) (inside Sin's valid
    range); DVE runs the Chebyshev recurrence s_k = 2 c1 s_{k-1} -
    s_{k-2} in fp16 (2x perf-mode tensor_tensor only).
  * cos identities: cos(2m t) = 1 - 2 s_m^2, cos((2i+1) t) = c1 -
    2 s_{i+1} s_i.  The Gram matmul therefore runs over RAW feature
    columns (bf16): one shared ones-column, and per batch element
    {c1, s_1^2..s_15^2, s_2 s_1, ..., s_16 s_15} (125 used + 3 zero
    pads).  Squares come from the otherwise-idle ScalarE (Square
    activation, stride-insensitive); odd products are single fp16
    tensor_tensor ops on DVE writing d-interleaved pairs (2x mode).
  * C layout col = (s*128 + c)*2 + d (s = sample column (jl n), c =
    function, d = dim) makes every matmul operand a 4-byte-stride AP
    (measured as fast as contiguous).  256 matmuls [128,128]x[128,128]
    bf16 accumulate into 2 alternating PSUM banks.
  * true S = A G A^T with sparse A (host, microseconds) + final loss.
  * u^2 on ScalarE (Square with accum_out); host sums the column.
"""

import math
from contextlib import ExitStack

import numpy as np

import concourse.bass as bass
import concourse.bacc as bacc
import concourse.mybir as mybir
import concourse.tile as tile
from concourse.bass_utils import run_bass_kernel_spmd

T, B, N, D, K = 512, 32, 64, 2, 32
NCORES = 8
BL = B // NCORES            # 4 batch elements per core
NT = N * T                  # 32768 samples per batch element
JJ = T // 128               # 4 t-chunks of 128 partitions
SCOL = 2 * N                # 128 sample columns (jl, n) per j-half
HCOLS = BL * SCOL * D       # 1024 x-columns per j-half (b, jl n, d)
NC = 128                    # function columns in the Gram
CTRL_SCALE = 1e-3 / (2.0 * N * T * B)
SAFETY = 1.0 - 1e-6         # keeps Sin's argument strictly inside [-pi, pi]

f32 = mybir.dt.float32
fp16 = mybir.dt.float16
bf16 = mybir.dt.bfloat16
ALU = mybir.AluOpType
ACTF = mybir.ActivationFunctionType

LAST_RESULTS = None         # stashed BassKernelResults for test harnesses


def colid(p, b):
    """Gram column index of cos-mode p for batch-slot b (device + host)."""
    if p == 0:
        return 0                      # shared ones column
    i = 1 + 31 * b
    if p == 1:
        return i                      # c1
    if p % 2 == 0:
        return i + p // 2             # s_m^2, m = p/2 in 1..15
    return i + 15 + (p - 1) // 2      # s_{i+1} s_i, i = (p-1)/2 in 1..15


def _build_body(ctx, tc, x_h, u_h, ga_h, gb_h, uc_h):
    nc = tc.nc

    xpool = ctx.enter_context(tc.tile_pool(name="xp", bufs=1))
    cpool = ctx.enter_context(tc.tile_pool(name="cp", bufs=1))
    spool = ctx.enter_context(tc.tile_pool(name="sp", bufs=6))
    qpool = ctx.enter_context(tc.tile_pool(name="qp", bufs=3))
    mpool = ctx.enter_context(tc.tile_pool(name="mp", bufs=1))
    ppool = ctx.enter_context(tc.tile_pool(name="pp", bufs=1, space="PSUM"))

    # ---- inputs to SBUF ----
    # x[t, b, n, d] -> X_h[p = t%128, (b (jl n) d)] for the two j-halves
    xv = x_h[:].rearrange("(j p) b n d -> p b j (n d)", j=JJ, p=128)
    Xh = []
    for h in range(2):
        X = xpool.tile([128, HCOLS], f32, tag=f"x{h}")
        nc.sync.dma_start(
            X[:].rearrange("p (b jl nd) -> p b jl nd", b=BL, jl=2, nd=N * D),
            xv[:, :, 2 * h : 2 * h + 2, :],
        )
        Xh.append(X)

    U = xpool.tile([128, 2048], f32, tag="u")
    nc.sync.dma_start(U[:], u_h[:].rearrange("(p a) b n d -> p (a b n d)", p=128))

    sc = mpool.tile([128, 8], f32, tag="scratch")
    bias_c1 = sc[:, 0:1]
    nc.vector.memset(bias_c1, float(np.float32(math.pi / 2 * SAFETY)))

    # u^2 summed per partition on DVE (early: fills the DMA-wait window)
    udum = mpool.tile([128, 2048], f32, tag="udum")
    ucol = sc[:, 1:2]
    nc.vector.tensor_mul(udum[:], U[:], U[:])
    nc.vector.tensor_reduce(ucol, udum[:], mybir.AxisListType.X, ALU.add)
    nc.sync.dma_start(uc_h[:], ucol)

    # ---- feature-column tensors: C_h[p, (s c d)], bf16 ----
    Ch = []
    for h in range(2):
        C = cpool.tile([128, NC * SCOL * D], bf16, tag=f"c{h}")
        CW = C[:].rearrange("p (s c d) -> p c s d", s=SCOL, c=NC, d=D)
        nc.vector.memset(CW[:, 0], 1.0)               # shared ones column
        nc.vector.memset(CW[:, 125:128], 0.0)         # zero pads
        Ch.append(C)

    Ga = ppool.tile([128, 128], f32, tag="ga")
    Gb = ppool.tile([128, 128], f32, tag="gb")
    mma = 0
    mmb = 0
    for h in range(2):
        X, C = Xh[h], Ch[h]

        # per-b column-family view: [p, i(31), b, s, d] for c = 1 + 31 b + i
        CF = C[:].rearrange("p (s c d) -> p s c d", s=SCOL, c=NC, d=D)
        CF = CF[:, :, 1:125, :].rearrange("p s (b i) d -> p i b s d", b=BL, i=31)

        def fcol(i):
            return CF[:, i]           # [128, b, s, d]

        Xin = X[:].rearrange("p (b s d) -> p b s d", b=BL, s=SCOL, d=D)

        def s_in(t):
            return t[:].rearrange("p (b s d) -> p b s d", b=BL, s=SCOL, d=D)

        # c1: fp16 tile for the chain + bf16 columns (both on ACT)
        c1 = qpool.tile([128, HCOLS], fp16, tag="c1")
        nc.scalar.activation(c1[:], X[:], ACTF.Sin,
                             bias=bias_c1, scale=float(np.float32(-math.pi * SAFETY)))
        nc.vector.tensor_copy(fcol(0), c1[:].rearrange(
            "p (b s d) -> p b s d", b=BL, s=SCOL, d=D))

        s_prev = spool.tile([128, HCOLS], fp16, tag="s")   # s_1
        nc.scalar.activation(s_prev[:], X[:], ACTF.Sin,
                             bias=0.0, scale=float(np.float32(math.pi * SAFETY)))

        c1d = qpool.tile([128, HCOLS], fp16, tag="c1d")    # 2*c1
        nc.vector.tensor_scalar_mul(c1d[:], c1[:], 2.0)

        # s_2 = 2 s_1 c_1 ; then per mode: squares on ACT, products on DVE
        s_cur = spool.tile([128, HCOLS], fp16, tag="s")
        nc.vector.tensor_mul(s_cur[:], s_prev[:], c1d[:])
        nc.vector.tensor_mul(fcol(1), s_in(s_prev), s_in(s_prev))    # s_1^2
        nc.vector.tensor_mul(fcol(16), s_in(s_cur), s_in(s_prev))    # s_2 s_1
        s_prev2, s_prev = s_prev, s_cur

        for m in range(3, 17):
            # s_m = 2 c1 s_{m-1} - s_{m-2}
            q = qpool.tile([128, HCOLS], fp16, tag="q")
            nc.vector.tensor_mul(q[:], s_prev[:], c1d[:])
            s_cur = spool.tile([128, HCOLS], fp16, tag="s")
            nc.vector.tensor_sub(s_cur[:], q[:], s_prev2[:])
            if m - 1 <= 15:
                nc.vector.tensor_mul(fcol(m - 1), s_in(s_prev), s_in(s_prev))
            nc.vector.tensor_mul(fcol(15 + m - 1), s_in(s_cur), s_in(s_prev))
            s_prev2, s_prev = s_prev, s_cur

        # Gram matmuls: one per sample column, alternating PSUM banks
        CM = C[:].rearrange("p (s c d) -> p s d c", s=SCOL, c=NC, d=D)
        for s_i in range(SCOL):
            if (s_i % 2) == 0:
                nc.tensor.matmul(Ga[:, :], CM[:, s_i, 0], CM[:, s_i, 1],
                                 start=(mma == 0), stop=(mma == JJ * N // 2 - 1))
                mma += 1
            else:
                nc.tensor.matmul(Gb[:, :], CM[:, s_i, 0], CM[:, s_i, 1],
                                 start=(mmb == 0), stop=(mmb == JJ * N // 2 - 1))
                mmb += 1

    # ---- outputs ----
    ga_sb = mpool.tile([128, 128], f32, tag="gasb")
    gb_sb = mpool.tile([128, 128], f32, tag="gbsb")
    nc.vector.tensor_copy(ga_sb[:], Ga[:, :])
    nc.vector.tensor_copy(gb_sb[:], Gb[:, :])
    nc.sync.dma_start(ga_h[:], ga_sb[:])
    nc.sync.dma_start(gb_h[:], gb_sb[:])


def _build_nc():
    nc = bacc.Bacc()
    x_h = nc.declare_dram_parameter("x", [T, BL, N, D], f32, isOutput=False)
    u_h = nc.declare_dram_parameter("u", [T, BL, N, D], f32, isOutput=False)
    ga_h = nc.declare_dram_parameter("ga", [128, 128], f32, isOutput=True)
    gb_h = nc.declare_dram_parameter("gb", [128, 128], f32, isOutput=True)
    uc_h = nc.declare_dram_parameter("uc", [128, 1], f32, isOutput=True)
    with tile.TileContext(nc) as tc:
        with ExitStack() as ctx:
            _build_body(ctx, tc, x_h, u_h, ga_h, gb_h, uc_h)
    nc.finalize()
    return nc


_NC_CACHE = None


def _get_nc():
    global _NC_CACHE
    if _NC_CACHE is None:
        _NC_CACHE = _build_nc()
    return _NC_CACHE


def _amat(b):
    """A[p, col]: cos-mode p as a linear combo of raw Gram columns."""
    A = np.zeros((K, NC), np.float32)
    for p in range(K):
        if p == 0:
            A[p, 0] = 1.0
        elif p == 1:
            A[p, colid(1, b)] = 1.0
        elif p % 2 == 0:
            A[p, colid(p, b)] = -2.0
            A[p, 0] += 1.0                     # + ones
        else:
            A[p, colid(p, b)] = -2.0
            A[p, colid(1, b)] += 1.0           # + c1
    return A


_AMATS = [_amat(b) for b in range(BL)]


def host_loss(gs, ucols, coeffs_density, norm_factors, norm_weights):
    nf = np.asarray(norm_factors, np.float32)
    cd = np.asarray(coeffs_density, np.float32)
    nw = np.asarray(norm_weights, np.float32)
    total = np.float32(0.0)
    for G, ucol in zip(gs, ucols):
        for b in range(BL):
            A = _AMATS[b]
            Sp = (A @ G @ A.T).astype(np.float32)
            coeffs = Sp / (nf * np.float32(NT))
            total = np.float32(
                total + (((coeffs - cd) ** 2) * nw).sum(dtype=np.float32))
        total = np.float32(
            total + np.float32(CTRL_SCALE) * ucol.sum(dtype=np.float32))
    return np.float32(total)


def make_in_maps(x, u):
    x = np.ascontiguousarray(np.asarray(x, dtype=np.float32))
    u = np.ascontiguousarray(np.asarray(u, dtype=np.float32))
    in_maps = []
    for c in range(NCORES):
        in_maps.append({
            "x": np.ascontiguousarray(x[:, BL * c : BL * (c + 1)]),
            "u": np.ascontiguousarray(u[:, BL * c : BL * (c + 1)]),
        })
    return in_maps


def kernel(x, u, L, coeffs_density, norm_factors, norm_weights):
    global LAST_RESULTS
    nc = _get_nc()
    in_maps = make_in_maps(x, u)
    res = run_bass_kernel_spmd(nc, in_maps, list(range(NCORES)))
    LAST_RESULTS = res
    gs = [np.asarray(r["ga"], np.float32) + np.asarray(r["gb"], np.float32)
          for r in res.results]
    ucols = [np.asarray(r["uc"], np.float32) for r in res.results]
    return host_loss(gs, ucols, coeffs_density, norm_factors, norm_weights)
